# revision 5
# baseline (speedup 1.0000x reference)
"""Attention-pooling kernel (AttLayer) for Trainium2, 8 NeuronCores.

Math (per batch b):
    uit  = tanh(x @ W + b)          # [T, A]
    ait  = exp(uit @ u)             # [T]
    out  = (sum_t ait[t] * x[t,:]) / (sum_t ait[t] + EPS)   # [D]

Single pass over x: accumulate numerator and denominator together.

Device data layout (per core, pure data-parallel over batch):
    xt   [2, 128, BL*T] f32   -- host-pre-transposed x: (d_chunk, d_in_chunk, b*t)
    w    [2, 128, A]    f32   -- W split into two 128-row d-chunks
    bb   [A, 1]         f32   -- bias, per-partition for ACT
    urep [A, 128]       f32   -- u broadcast to 128 columns (logit-broadcast trick)
Outputs:
    num  [128, 2*BL]    f32   -- numerator, col = b*2 + c
    den  [1, NQ*BL]     f32   -- per-quarter exp-sum partials, col = b*NQ + h*2 + q

Pipeline per (b, t-quarter):
    PE : psum_uitT[A, TQ]   = W0^T @ xt0 + W1^T @ xt1        (contract d)
    ACT: uitT_sb            = tanh(psum_uitT + bb)           (bias per-partition)
    PE : psum_logit[128,TQ] = urep^T @ uitT_sb               (contract a; rows identical)
    ACT: e_sb               = exp(psum_logit), accum_out -> den partial
    DVE: tensor_tensor_reduce(xt_c * e_sb) -> num[:, b*2+c]  (fused mul+reduce, chained)
"""

import sys

sys.path.insert(0, "/opt/trn_rl_repo")

import numpy as np

import concourse.bacc as bacc
import concourse.tile as tile
from concourse import mybir
from concourse import bass_utils
from concourse.dve_ops import TENSOR_TENSOR_REDUCE

B, T, D, A = 64, 4096, 256, 50
NCORES = 8
BL = B // NCORES  # batches per core
EPS = 1e-7
P = 128
NCH = D // P  # 2 d-chunks


def build_attpool(nc, aps, BL, T):
    """Emit the tile program. aps: dict name->AP for dram tensors."""
    TH = T // 2   # half (TTR granularity)
    TQ = T // 4   # quarter (ACT/psum granularity)
    NQ = 4
    xt, w, bb, urep = aps["xt"], aps["w"], aps["bb"], aps["urep"]
    num, den = aps["num"], aps["den"]
    f32 = mybir.dt.float32

    with tile.TileContext(nc) as tc:
        with (
            tc.tile_pool(name="singles", bufs=1) as singles,
            tc.tile_pool(name="x0", bufs=2) as x0_pool,
            tc.tile_pool(name="x1", bufs=2) as x1_pool,
            tc.tile_pool(name="uitT", bufs=2) as uitT_pool,
            tc.tile_pool(name="e", bufs=2) as e_pool,
            tc.tile_pool(name="scratch", bufs=2) as scratch_pool,
            tc.tile_pool(name="ps_uitT", bufs=2, space="PSUM") as ps_uitT_pool,
            tc.tile_pool(name="ps_logit", bufs=2, space="PSUM") as ps_logit_pool,
        ):
            # constants
            w_sb = [
                singles.tile([P, A], f32, tag=f"w{c}", name=f"w_sb{c}")
                for c in range(NCH)
            ]
            for c in range(NCH):
                nc.sync.dma_start(out=w_sb[c][:, :], in_=w[c, :, :])
            bb_sb = singles.tile([A, 1], f32)
            nc.sync.dma_start(out=bb_sb[:, :], in_=bb[:, :])
            urep_sb = singles.tile([A, P], f32)
            nc.sync.dma_start(out=urep_sb[:, :], in_=urep[:, :])
            num_sb = singles.tile([P, NCH * BL], f32)
            den_sb = singles.tile([P, NQ * BL], f32)

            npairs = (BL + 1) // 2
            for p in range(npairs):
                nb = min(2, BL - 2 * p)  # batches in this pair-tile
                xt_t = [None, None]
                for c, pool in ((0, x0_pool), (1, x1_pool)):
                    xt_t[c] = pool.tile([P, 2 * T], f32, tag=f"xt{c}", name=f"xt_t{c}")
                    nc.sync.dma_start(
                        out=xt_t[c][:, : nb * T],
                        in_=xt[c, :, 2 * p * T : (2 * p + nb) * T],
                    )
                for bi in range(nb):
                    b = 2 * p + bi
                    for h in range(2):
                        e_sb = e_pool.tile([P, TH], f32, tag="e")
                        for q in range(2):
                            qi = h * 2 + q          # quarter index in batch
                            off = bi * T + qi * TQ  # col offset in xt tile
                            ps_uitT = ps_uitT_pool.tile([A, TQ], f32, tag="psu")
                            # step 1: uitT = W^T @ xT  (contract d, 512-col blocks)
                            for s in range(0, TQ, 512):
                                sw = min(512, TQ - s)
                                for c in range(NCH):
                                    nc.tensor.matmul(
                                        ps_uitT[:, s : s + sw],
                                        lhsT=w_sb[c][:, :],
                                        rhs=xt_t[c][:, off + s : off + s + sw],
                                        start=(c == 0),
                                        stop=(c == NCH - 1),
                                    )
                            # tanh(+bias) -> sbuf
                            uitT_sb = uitT_pool.tile([A, TQ], f32, tag="uitT")
                            nc.scalar.activation(
                                uitT_sb[:, :], ps_uitT[:, :],
                                mybir.ActivationFunctionType.Tanh,
                                bias=bb_sb[:, :],
                            )
                            # step 2: logits broadcast to 128 partitions
                            ps_logit = ps_logit_pool.tile([P, TQ], f32, tag="psl")
                            for s in range(0, TQ, 512):
                                sw = min(512, TQ - s)
                                nc.tensor.matmul(
                                    ps_logit[:, s : s + sw],
                                    lhsT=urep_sb[:, :],
                                    rhs=uitT_sb[:, s : s + sw],
                                    start=True,
                                    stop=True,
                                )
                            # exp -> e_sb, denominator partial
                            nc.scalar.activation(
                                e_sb[:, q * TQ : (q + 1) * TQ], ps_logit[:, :],
                                mybir.ActivationFunctionType.Exp,
                                accum_out=den_sb[:, b * NQ + qi : b * NQ + qi + 1],
                            )
                        # step 4: fused multiply+reduce over this half
                        for c in range(NCH):
                            col = b * NCH + c
                            scr = scratch_pool.tile([P, TH], f32, tag="scr")
                            nc.vector._custom_dve(
                                TENSOR_TENSOR_REDUCE,
                                out=scr[:, :],
                                in0=xt_t[c][:, bi * T + h * TH : bi * T + (h + 1) * TH],
                                in1=e_sb[:, :],
                                s0=0.0 if h == 0 else num_sb[:, col : col + 1],
                                s1=1.0,
                                accum_out=num_sb[:, col : col + 1],
                            )
            nc.sync.dma_start(out=num[:, :], in_=num_sb[:, :])
            nc.sync.dma_start(out=den[:, :], in_=den_sb[0:1, :])
    return nc


def _declare(nc, BL, T):
    f32 = mybir.dt.float32
    NQ = 4
    aps = {
        "xt": nc.dram_tensor("xt", (NCH, P, BL * T), f32, kind="ExternalInput").ap(),
        "w": nc.dram_tensor("w", (NCH, P, A), f32, kind="ExternalInput").ap(),
        "bb": nc.dram_tensor("bb", (A, 1), f32, kind="ExternalInput").ap(),
        "urep": nc.dram_tensor("urep", (A, P), f32, kind="ExternalInput").ap(),
        "num": nc.dram_tensor("num", (P, NCH * BL), f32, kind="ExternalOutput").ap(),
        "den": nc.dram_tensor("den", (1, NQ * BL), f32, kind="ExternalOutput").ap(),
    }
    return aps


_CACHE = {}


def _get_nc():
    key = "nc"
    if key not in _CACHE:
        nc = bacc.Bacc(
            "TRN2", target_bir_lowering=False, debug=False,
            enable_asserts=False, num_devices=NCORES,
        )
        aps = _declare(nc, BL, T)
        build_attpool(nc, aps, BL, T)
        nc.compile()
        _CACHE[key] = nc
    return _CACHE[key]


def _host_prep(x, W, b, u):
    """Build per-core input maps from full inputs."""
    x = np.asarray(x, dtype=np.float32)
    W = np.asarray(W, dtype=np.float32)
    b = np.asarray(b, dtype=np.float32)
    u = np.asarray(u, dtype=np.float32)
    wc = np.ascontiguousarray(W.reshape(NCH, P, A))
    bb = np.ascontiguousarray(b.reshape(A, 1))
    urep = np.ascontiguousarray(np.tile(u.reshape(A, 1), (1, P)))
    in_maps = []
    for core in range(NCORES):
        xc = x[core * BL : (core + 1) * BL]  # [BL, T, D]
        # -> [NCH, P, BL*T]: xt[c, dp, b*T+t] = x[b, t, c*128+dp]
        xt = np.ascontiguousarray(
            xc.reshape(BL, T, NCH, P).transpose(2, 3, 0, 1).reshape(NCH, P, BL * T)
        )
        in_maps.append({"xt": xt, "w": wc, "bb": bb, "urep": urep})
    return in_maps


def _unshard(results):
    out = np.empty((B, D), dtype=np.float32)
    NQ = 4
    for core in range(NCORES):
        num = results[core]["num"]          # [128, 2*BL]
        den = results[core]["den"]          # [1, NQ*BL]
        den_b = den.reshape(BL, NQ).sum(axis=1)  # [BL]
        for bl in range(BL):
            vec = np.concatenate(
                [num[:, bl * NCH + c] for c in range(NCH)]
            )  # [D]
            out[core * BL + bl] = vec / (den_b[bl] + EPS)
    return out


def kernel(x, W, b, u, _trace=False):
    nc = _get_nc()
    in_maps = _host_prep(x, W, b, u)
    res = bass_utils.run_bass_kernel_spmd(
        nc, in_maps, core_ids=list(range(NCORES)), trace=_trace,
    )
    out = _unshard(res.results)
    if _trace:
        kernel.last_result = res
    return out


# revision 6
# speedup vs baseline: 2.2916x; 2.2916x over previous
"""Attention-pooling kernel (AttLayer) for Trainium2, 8 NeuronCores.

Math (per batch b):
    uit  = tanh(x @ W + b)          # [T, A]
    ait  = exp(uit @ u)             # [T]
    out  = (sum_t ait[t] * x[t,:]) / (sum_t ait[t] + EPS)   # [D]

Single pass over x: accumulate numerator and denominator together.

Device data layout (per core, pure data-parallel over batch):
    xt   [2, 128, BL*T] f32   -- host-pre-transposed x: (d_chunk, d_in_chunk, b*t)
    w    [2, 128, A]    f32   -- W split into two 128-row d-chunks
    bb   [A, 1]         f32   -- bias, per-partition for ACT
    urep [A, 128]       f32   -- u broadcast to 128 columns (logit-broadcast trick)
Outputs:
    num  [128, 2*BL]    f32   -- numerator, col = b*2 + c
    den  [1, NQ*BL]     f32   -- per-quarter exp-sum partials, col = b*NQ + h*2 + q

Pipeline per (b, t-quarter):
    PE : psum_uitT[A, TQ]   = W0^T @ xt0 + W1^T @ xt1        (contract d)
    ACT: uitT_sb            = tanh(psum_uitT + bb)           (bias per-partition)
    PE : psum_logit[128,TQ] = urep^T @ uitT_sb               (contract a; rows identical)
    ACT: e_sb               = exp(psum_logit), accum_out -> den partial
    DVE: tensor_tensor_reduce(xt_c * e_sb) -> num[:, b*2+c]  (fused mul+reduce, chained)
"""

import sys

sys.path.insert(0, "/opt/trn_rl_repo")

import numpy as np
import ml_dtypes

import concourse.bacc as bacc
import concourse.tile as tile
from concourse import mybir
from concourse import bass_utils
from concourse.dve_ops import TENSOR_TENSOR_REDUCE

B, T, D, A = 64, 4096, 256, 50
NCORES = 8
BL = B // NCORES  # batches per core
EPS = 1e-7
P = 128
NCH = D // P  # 2 d-chunks


def build_attpool(nc, aps, BL, T):
    """Emit the tile program. aps: dict name->AP for dram tensors."""
    TH = T // 2   # half (TTR granularity)
    TQ = T // 4   # quarter (ACT/psum granularity)
    NQ = 4
    xt, w, bb, urep = aps["xt"], aps["w"], aps["bb"], aps["urep"]
    num, den = aps["num"], aps["den"]
    f32 = mybir.dt.float32
    bf16 = mybir.dt.bfloat16

    with tile.TileContext(nc) as tc:
        with (
            tc.tile_pool(name="singles", bufs=1) as singles,
            tc.tile_pool(name="x0", bufs=2) as x0_pool,
            tc.tile_pool(name="x1", bufs=2) as x1_pool,
            tc.tile_pool(name="uitT", bufs=2) as uitT_pool,
            tc.tile_pool(name="e", bufs=2) as e_pool,
            tc.tile_pool(name="scratch", bufs=2) as scratch_pool,
            tc.tile_pool(name="ps_uitT", bufs=2, space="PSUM") as ps_uitT_pool,
            tc.tile_pool(name="ps_logit", bufs=2, space="PSUM") as ps_logit_pool,
        ):
            # constants
            w_sb = [
                singles.tile([P, A], bf16, tag=f"w{c}", name=f"w_sb{c}")
                for c in range(NCH)
            ]
            for c in range(NCH):
                nc.sync.dma_start(out=w_sb[c][:, :], in_=w[c, :, :])
            bb_sb = singles.tile([A, 1], f32)
            nc.sync.dma_start(out=bb_sb[:, :], in_=bb[:, :])
            urep_sb = singles.tile([A, P], bf16)
            nc.sync.dma_start(out=urep_sb[:, :], in_=urep[:, :])
            num_sb = singles.tile([P, NCH * BL], f32)
            den_sb = singles.tile([P, NQ * BL], f32)

            npairs = (BL + 1) // 2
            for p in range(npairs):
                nb = min(2, BL - 2 * p)  # batches in this pair-tile
                xt_t = [None, None]
                for c, pool in ((0, x0_pool), (1, x1_pool)):
                    xt_t[c] = pool.tile([P, 2 * T], bf16, tag=f"xt{c}", name=f"xt_t{c}")
                    nc.sync.dma_start(
                        out=xt_t[c][:, : nb * T],
                        in_=xt[c, :, 2 * p * T : (2 * p + nb) * T],
                    )
                for bi in range(nb):
                    b = 2 * p + bi
                    for h in range(2):
                        e_sb = e_pool.tile([P, TH], bf16, tag="e")
                        for q in range(2):
                            qi = h * 2 + q          # quarter index in batch
                            off = bi * T + qi * TQ  # col offset in xt tile
                            ps_uitT = ps_uitT_pool.tile([A, TQ], f32, tag="psu")
                            # step 1: uitT = W^T @ xT  (contract d, 512-col blocks)
                            for s in range(0, TQ, 512):
                                sw = min(512, TQ - s)
                                for c in range(NCH):
                                    nc.tensor.matmul(
                                        ps_uitT[:, s : s + sw],
                                        lhsT=w_sb[c][:, :],
                                        rhs=xt_t[c][:, off + s : off + s + sw],
                                        start=(c == 0),
                                        stop=(c == NCH - 1),
                                    )
                            # tanh(+bias) -> sbuf
                            uitT_sb = uitT_pool.tile([A, TQ], bf16, tag="uitT")
                            nc.scalar.activation(
                                uitT_sb[:, :], ps_uitT[:, :],
                                mybir.ActivationFunctionType.Tanh,
                                bias=bb_sb[:, :],
                            )
                            # step 2: logits broadcast to 128 partitions
                            ps_logit = ps_logit_pool.tile([P, TQ], f32, tag="psl")
                            for s in range(0, TQ, 512):
                                sw = min(512, TQ - s)
                                nc.tensor.matmul(
                                    ps_logit[:, s : s + sw],
                                    lhsT=urep_sb[:, :],
                                    rhs=uitT_sb[:, s : s + sw],
                                    start=True,
                                    stop=True,
                                )
                            # exp -> e_sb, denominator partial
                            nc.scalar.activation(
                                e_sb[:, q * TQ : (q + 1) * TQ], ps_logit[:, :],
                                mybir.ActivationFunctionType.Exp,
                                accum_out=den_sb[:, b * NQ + qi : b * NQ + qi + 1],
                            )
                        # step 4: fused multiply+reduce over this half
                        for c in range(NCH):
                            col = b * NCH + c
                            scr = scratch_pool.tile([P, TH], bf16, tag="scr")
                            nc.vector._custom_dve(
                                TENSOR_TENSOR_REDUCE,
                                out=scr[:, :],
                                in0=xt_t[c][:, bi * T + h * TH : bi * T + (h + 1) * TH],
                                in1=e_sb[:, :],
                                s0=0.0 if h == 0 else num_sb[:, col : col + 1],
                                s1=1.0,
                                accum_out=num_sb[:, col : col + 1],
                            )
            nc.sync.dma_start(out=num[:, :], in_=num_sb[:, :])
            nc.sync.dma_start(out=den[:, :], in_=den_sb[0:1, :])
    return nc


def _declare(nc, BL, T):
    f32 = mybir.dt.float32
    bf16 = mybir.dt.bfloat16
    NQ = 4
    aps = {
        "xt": nc.dram_tensor("xt", (NCH, P, BL * T), bf16, kind="ExternalInput").ap(),
        "w": nc.dram_tensor("w", (NCH, P, A), bf16, kind="ExternalInput").ap(),
        "bb": nc.dram_tensor("bb", (A, 1), f32, kind="ExternalInput").ap(),
        "urep": nc.dram_tensor("urep", (A, P), bf16, kind="ExternalInput").ap(),
        "num": nc.dram_tensor("num", (P, NCH * BL), f32, kind="ExternalOutput").ap(),
        "den": nc.dram_tensor("den", (1, NQ * BL), f32, kind="ExternalOutput").ap(),
    }
    return aps


_CACHE = {}


def _get_nc():
    key = "nc"
    if key not in _CACHE:
        nc = bacc.Bacc(
            "TRN2", target_bir_lowering=False, debug=False,
            enable_asserts=False, num_devices=NCORES,
        )
        aps = _declare(nc, BL, T)
        build_attpool(nc, aps, BL, T)
        nc.compile()
        _CACHE[key] = nc
    return _CACHE[key]


def _host_prep(x, W, b, u):
    """Build per-core input maps from full inputs."""
    x = np.asarray(x, dtype=np.float32)
    W = np.asarray(W, dtype=np.float32)
    b = np.asarray(b, dtype=np.float32)
    u = np.asarray(u, dtype=np.float32)
    wc = np.ascontiguousarray(W.reshape(NCH, P, A)).astype(ml_dtypes.bfloat16)
    bb = np.ascontiguousarray(b.reshape(A, 1))
    urep = np.ascontiguousarray(np.tile(u.reshape(A, 1), (1, P))).astype(ml_dtypes.bfloat16)
    in_maps = []
    for core in range(NCORES):
        xc = x[core * BL : (core + 1) * BL]  # [BL, T, D]
        # -> [NCH, P, BL*T]: xt[c, dp, b*T+t] = x[b, t, c*128+dp]
        xt = np.ascontiguousarray(
            xc.reshape(BL, T, NCH, P).transpose(2, 3, 0, 1).reshape(NCH, P, BL * T)
        ).astype(ml_dtypes.bfloat16)
        in_maps.append({"xt": xt, "w": wc, "bb": bb, "urep": urep})
    return in_maps


def _unshard(results):
    out = np.empty((B, D), dtype=np.float32)
    NQ = 4
    for core in range(NCORES):
        num = results[core]["num"]          # [128, 2*BL]
        den = results[core]["den"]          # [1, NQ*BL]
        den_b = den.reshape(BL, NQ).sum(axis=1)  # [BL]
        for bl in range(BL):
            vec = np.concatenate(
                [num[:, bl * NCH + c] for c in range(NCH)]
            )  # [D]
            out[core * BL + bl] = vec / (den_b[bl] + EPS)
    return out


def kernel(x, W, b, u, _trace=False):
    nc = _get_nc()
    in_maps = _host_prep(x, W, b, u)
    res = bass_utils.run_bass_kernel_spmd(
        nc, in_maps, core_ids=list(range(NCORES)), trace=_trace,
    )
    out = _unshard(res.results)
    if _trace:
        kernel.last_result = res
    return out


# revision 7
# speedup vs baseline: 2.3512x; 1.0260x over previous
"""Attention-pooling kernel (AttLayer) for Trainium2, 8 NeuronCores.

Math (per batch b):
    uit  = tanh(x @ W + b)          # [T, A]
    ait  = exp(uit @ u)             # [T]
    out  = (sum_t ait[t] * x[t,:]) / (sum_t ait[t] + EPS)   # [D]

Single pass over x: accumulate numerator and denominator together.

Device data layout (per core, pure data-parallel over batch):
    xt   [2, 128, BL*T] f32   -- host-pre-transposed x: (d_chunk, d_in_chunk, b*t)
    w    [2, 128, A]    f32   -- W split into two 128-row d-chunks
    bb   [A, 1]         f32   -- bias, per-partition for ACT
    urep [A, 128]       f32   -- u broadcast to 128 columns (logit-broadcast trick)
Outputs:
    num  [128, 2*BL]    f32   -- numerator, col = b*2 + c
    den  [1, NQ*BL]     f32   -- per-quarter exp-sum partials, col = b*NQ + h*2 + q

Pipeline per (b, t-quarter):
    PE : psum_uitT[A, TQ]   = W0^T @ xt0 + W1^T @ xt1        (contract d)
    ACT: uitT_sb            = tanh(psum_uitT + bb)           (bias per-partition)
    PE : psum_logit[128,TQ] = urep^T @ uitT_sb               (contract a; rows identical)
    ACT: e_sb               = exp(psum_logit), accum_out -> den partial
    DVE: tensor_tensor_reduce(xt_c * e_sb) -> num[:, b*2+c]  (fused mul+reduce, chained)
"""

import sys

sys.path.insert(0, "/opt/trn_rl_repo")

import numpy as np
import ml_dtypes

import concourse.bacc as bacc
import concourse.tile as tile
from concourse import mybir
from concourse import bass_utils
from concourse.dve_ops import TENSOR_TENSOR_REDUCE

B, T, D, A = 64, 4096, 256, 50
NCORES = 8
BL = B // NCORES  # batches per core
EPS = 1e-7
P = 128
NCH = D // P  # 2 d-chunks


def build_attpool(nc, aps, BL, T):
    """Emit the tile program. aps: dict name->AP for dram tensors."""
    TH = T // 2   # half (TTR granularity)
    TQ = T // 4   # quarter (ACT/psum granularity)
    NQ = 4
    xt, w, bb, urep = aps["xt"], aps["w"], aps["bb"], aps["urep"]
    num, den = aps["num"], aps["den"]
    f32 = mybir.dt.float32
    bf16 = mybir.dt.bfloat16

    with tile.TileContext(nc) as tc:
        with (
            tc.tile_pool(name="singles", bufs=1) as singles,
            tc.tile_pool(name="x0", bufs=2) as x0_pool,
            tc.tile_pool(name="x1", bufs=2) as x1_pool,
            tc.tile_pool(name="uitT", bufs=2) as uitT_pool,
            tc.tile_pool(name="e", bufs=2) as e_pool,
            tc.tile_pool(name="scratch", bufs=2) as scratch_pool,
            tc.tile_pool(name="ps_uitT", bufs=2, space="PSUM") as ps_uitT_pool,
            tc.tile_pool(name="ps_logit", bufs=2, space="PSUM") as ps_logit_pool,
        ):
            # constants
            w_sb = [
                singles.tile([P, A], bf16, tag=f"w{c}", name=f"w_sb{c}")
                for c in range(NCH)
            ]
            for c in range(NCH):
                nc.sync.dma_start(out=w_sb[c][:, :], in_=w[c, :, :])
            bb_sb = singles.tile([A, 1], f32)
            nc.sync.dma_start(out=bb_sb[:, :], in_=bb[:, :])
            urep_sb = singles.tile([A, P], bf16)
            nc.sync.dma_start(out=urep_sb[:, :], in_=urep[:, :])
            num_sb = singles.tile([P, NCH * BL], f32)
            den_sb = singles.tile([P, NQ * BL], f32)

            npairs = (BL + 1) // 2

            # flatten all quarters; software-pipeline by one stage so PE's
            # step1(i+1) is emitted before step2(i): keeps PE/ACT streaming
            # instead of ping-ponging on the s1->tanh->s2->exp chain.
            quarters = []
            for p in range(npairs):
                nb = min(2, BL - 2 * p)
                for bi in range(nb):
                    for h in range(2):
                        for q in range(2):
                            quarters.append((p, nb, bi, h, q))

            xt_tiles = {}   # pair -> [tile_c0, tile_c1]
            e_tiles = {}    # (b, h) -> e_sb tile

            def load_pair(p, nb):
                xt_t = [None, None]
                for c, pool in ((0, x0_pool), (1, x1_pool)):
                    xt_t[c] = pool.tile(
                        [P, 2 * T], bf16, tag=f"xt{c}", name=f"xt_t{c}"
                    )
                    nc.sync.dma_start(
                        out=xt_t[c][:, : nb * T],
                        in_=xt[c, :, 2 * p * T : (2 * p + nb) * T],
                    )
                xt_tiles[p] = xt_t

            def stage1(p, nb, bi, h, q):
                if p not in xt_tiles:
                    load_pair(p, nb)
                xt_t = xt_tiles[p]
                qi = h * 2 + q
                off = bi * T + qi * TQ
                ps_uitT = ps_uitT_pool.tile([A, TQ], f32, tag="psu")
                # W0 for both 512-blocks, then W1: adjacent same-weight MMs
                for c in range(NCH):
                    for s in range(0, TQ, 512):
                        sw = min(512, TQ - s)
                        nc.tensor.matmul(
                            ps_uitT[:, s : s + sw],
                            lhsT=w_sb[c][:, :],
                            rhs=xt_t[c][:, off + s : off + s + sw],
                            start=(c == 0),
                            stop=(c == NCH - 1),
                        )
                return ps_uitT

            def stage2(state):
                (p, nb, bi, h, q), ps_uitT = state
                b = 2 * p + bi
                qi = h * 2 + q
                uitT_sb = uitT_pool.tile([A, TQ], bf16, tag="uitT")
                nc.scalar.activation(
                    uitT_sb[:, :], ps_uitT[:, :],
                    mybir.ActivationFunctionType.Tanh,
                    bias=bb_sb[:, :],
                )
                ps_logit = ps_logit_pool.tile([P, TQ], f32, tag="psl")
                for s in range(0, TQ, 512):
                    sw = min(512, TQ - s)
                    nc.tensor.matmul(
                        ps_logit[:, s : s + sw],
                        lhsT=urep_sb[:, :],
                        rhs=uitT_sb[:, s : s + sw],
                        start=True,
                        stop=True,
                    )
                if (b, h) not in e_tiles:
                    e_tiles[(b, h)] = e_pool.tile([P, TH], bf16, tag="e", name="e_sb")
                e_sb = e_tiles[(b, h)]
                nc.scalar.activation(
                    e_sb[:, q * TQ : (q + 1) * TQ], ps_logit[:, :],
                    mybir.ActivationFunctionType.Exp,
                    accum_out=den_sb[:, b * NQ + qi : b * NQ + qi + 1],
                )
                if q == 1:
                    # half complete: fused multiply+reduce on DVE
                    xt_t = xt_tiles[p]
                    for c in range(NCH):
                        col = b * NCH + c
                        scr = scratch_pool.tile([P, TH], bf16, tag="scr")
                        nc.vector._custom_dve(
                            TENSOR_TENSOR_REDUCE,
                            out=scr[:, :],
                            in0=xt_t[c][:, bi * T + h * TH : bi * T + (h + 1) * TH],
                            in1=e_sb[:, :],
                            s0=0.0 if h == 0 else num_sb[:, col : col + 1],
                            s1=1.0,
                            accum_out=num_sb[:, col : col + 1],
                        )
                    del e_tiles[(b, h)]

            pend = None
            for qd in quarters:
                ps = stage1(*qd)
                if pend is not None:
                    stage2(pend)
                pend = (qd, ps)
            stage2(pend)
            nc.sync.dma_start(out=num[:, :], in_=num_sb[:, :])
            nc.sync.dma_start(out=den[:, :], in_=den_sb[0:1, :])
    return nc


def _declare(nc, BL, T):
    f32 = mybir.dt.float32
    bf16 = mybir.dt.bfloat16
    NQ = 4
    aps = {
        "xt": nc.dram_tensor("xt", (NCH, P, BL * T), bf16, kind="ExternalInput").ap(),
        "w": nc.dram_tensor("w", (NCH, P, A), bf16, kind="ExternalInput").ap(),
        "bb": nc.dram_tensor("bb", (A, 1), f32, kind="ExternalInput").ap(),
        "urep": nc.dram_tensor("urep", (A, P), bf16, kind="ExternalInput").ap(),
        "num": nc.dram_tensor("num", (P, NCH * BL), f32, kind="ExternalOutput").ap(),
        "den": nc.dram_tensor("den", (1, NQ * BL), f32, kind="ExternalOutput").ap(),
    }
    return aps


_CACHE = {}


def _get_nc():
    key = "nc"
    if key not in _CACHE:
        nc = bacc.Bacc(
            "TRN2", target_bir_lowering=False, debug=False,
            enable_asserts=False, num_devices=NCORES,
        )
        aps = _declare(nc, BL, T)
        build_attpool(nc, aps, BL, T)
        nc.compile()
        _CACHE[key] = nc
    return _CACHE[key]


def _host_prep(x, W, b, u):
    """Build per-core input maps from full inputs."""
    x = np.asarray(x, dtype=np.float32)
    W = np.asarray(W, dtype=np.float32)
    b = np.asarray(b, dtype=np.float32)
    u = np.asarray(u, dtype=np.float32)
    wc = np.ascontiguousarray(W.reshape(NCH, P, A)).astype(ml_dtypes.bfloat16)
    bb = np.ascontiguousarray(b.reshape(A, 1))
    urep = np.ascontiguousarray(np.tile(u.reshape(A, 1), (1, P))).astype(ml_dtypes.bfloat16)
    in_maps = []
    for core in range(NCORES):
        xc = x[core * BL : (core + 1) * BL]  # [BL, T, D]
        # -> [NCH, P, BL*T]: xt[c, dp, b*T+t] = x[b, t, c*128+dp]
        xt = np.ascontiguousarray(
            xc.reshape(BL, T, NCH, P).transpose(2, 3, 0, 1).reshape(NCH, P, BL * T)
        ).astype(ml_dtypes.bfloat16)
        in_maps.append({"xt": xt, "w": wc, "bb": bb, "urep": urep})
    return in_maps


def _unshard(results):
    out = np.empty((B, D), dtype=np.float32)
    NQ = 4
    for core in range(NCORES):
        num = results[core]["num"]          # [128, 2*BL]
        den = results[core]["den"]          # [1, NQ*BL]
        den_b = den.reshape(BL, NQ).sum(axis=1)  # [BL]
        for bl in range(BL):
            vec = np.concatenate(
                [num[:, bl * NCH + c] for c in range(NCH)]
            )  # [D]
            out[core * BL + bl] = vec / (den_b[bl] + EPS)
    return out


def kernel(x, W, b, u, _trace=False):
    nc = _get_nc()
    in_maps = _host_prep(x, W, b, u)
    res = bass_utils.run_bass_kernel_spmd(
        nc, in_maps, core_ids=list(range(NCORES)), trace=_trace,
    )
    out = _unshard(res.results)
    if _trace:
        kernel.last_result = res
    return out
